# revision 12
# baseline (speedup 1.0000x reference)
"""Causal cross-attention (B=4, L=2048, D=1024, H=16, hd=64) on 8 trn2 cores.

Sharding: core c -> (batch b = c//2, head-group g = c%2 of 8 heads).
Each core computes QKV projections for its head group, causal-masked
per-head attention, and a partial output projection (its heads' columns
of Wo). Host sums the two partials per batch and adds bo.

On-chip layout is feature-major ("transposed") end to end so no on-chip
transposes are needed:
  QhT/KhT: [feat, seq], V: [seq, feat(+ones col)] -> scoresT = KhT_blk.T @ QhT_blk
  attn_outT accumulated as V_ext.T @ probsT with a ones column giving the
  softmax denominator for free; normalization via reciprocal + K=1
  broadcast matmul.

v2 scheduling notes:
  - Causal masking runs on the PE as a tiny bf16 matmul (identity x
    triangle) accumulated into the scores PSUM group - the DVE is out of
    the scores->exp critical path entirely.
  - Diagonal kv chunks compute only the live q range (512/384/256/128
    wide); qh and the probs are bf16 so narrow matmuls still stream at
    1 cycle/row.
  - PSUM->SBUF staging copies (attention out, o-proj out, v bias) run on
    the otherwise idle GPSIMD/Pool engine.
  - Phase 1 (K/V projection) walks the contraction dim chunk-major with
    8 PSUM accumulator banks so compute starts after the first 512-col
    chunk of wk/kv arrives; DMAs are issued in consumption order.
"""

import numpy as np
import ml_dtypes

B, L, D, H, HD = 4, 2048, 1024, 16, 64
NCORES = 8
SCALE = HD ** -0.5

_CACHE = {}


def _build_nc():
    import concourse.mybir as mybir
    import concourse.tile as tile
    from concourse import bacc

    F32 = mybir.dt.float32
    F32R = mybir.dt.float32r
    BF16 = mybir.dt.bfloat16
    AF = mybir.ActivationFunctionType
    ALU = mybir.AluOpType

    nc = bacc.Bacc("TRN2", target_bir_lowering=False, debug=False)

    qt_d = nc.declare_dram_parameter("qt", [4, 128, 4096], BF16, isOutput=False)
    kvt_d = nc.declare_dram_parameter("kvt", [4, 128, 4096], BF16, isOutput=False)
    wq_d = nc.declare_dram_parameter("wq", [128, 4096], BF16, isOutput=False)
    wk_d = nc.declare_dram_parameter("wk", [128, 4096], BF16, isOutput=False)
    wv_d = nc.declare_dram_parameter("wv", [128, 4096], BF16, isOutput=False)
    wo_d = nc.declare_dram_parameter("wo", [128, 4096], BF16, isOutput=False)
    bq_d = nc.declare_dram_parameter("bq", [128, 4], F32, isOutput=False)
    bk_d = nc.declare_dram_parameter("bk", [128, 4], F32, isOutput=False)
    bv_d = nc.declare_dram_parameter("bv", [1, 512], F32R, isOutput=False)
    ones_d = nc.declare_dram_parameter("ones", [1, 128], F32R, isOutput=False)
    ones8_d = nc.declare_dram_parameter("ones8", [128, 8], F32R, isOutput=False)
    tri_d = nc.declare_dram_parameter("tri", [128, 128], BF16, isOutput=False)
    idn_d = nc.declare_dram_parameter("idn", [128, 128], BF16, isOutput=False)
    e2_d = nc.declare_dram_parameter("e2", [128, 128], F32R, isOutput=False)
    out_d = nc.declare_dram_parameter("out", [2048, 1024], F32, isOutput=True)

    with tile.TileContext(nc) as tc:
        with (
            tc.tile_pool(name="const", bufs=1) as const,
            tc.tile_pool(name="w", bufs=4) as wp,
            tc.tile_pool(name="stream", bufs=4) as stream,
            tc.tile_pool(name="khp", bufs=1) as khp,
            tc.tile_pool(name="vxp", bufs=1) as vxp,
            tc.tile_pool(name="qhp", bufs=2) as qhp,
            tc.tile_pool(name="atp", bufs=2) as atp,
            tc.tile_pool(name="prp", bufs=2) as prp,
            tc.tile_pool(name="smp", bufs=2) as smp,
            tc.tile_pool(name="osp", bufs=4) as osp,
        ):
            # consts needed by the very first PE op (bv broadcast) go first;
            # the rest of the small consts slot in behind the first two
            # wk/kv chunk loads so they don't delay compute start
            bq_t = const.tile([128, 4], F32, tag="bq")
            bk_t = const.tile([128, 4], F32, tag="bk")
            ones_t = const.tile([1, 128], F32R, tag="ones")
            ones8_t = const.tile([128, 8], F32R, tag="ones8")
            bv_t = const.tile([1, 512], F32R, tag="bv")
            wk_t = wp.tile([128, 4096], BF16, tag="w")
            wv_t = wp.tile([128, 4096], BF16, tag="w")
            ks0 = stream.tile([128, 4096], BF16, tag="stream", name="ks0")
            for g in range(4):
                s = slice(g * 1024, (g + 1) * 1024)
                nc.sync.dma_start(wk_t[:, s], wk_d[:, s])
                nc.sync.dma_start(ks0[:, s], kvt_d[0, :, s])
                if g == 0:
                    nc.sync.dma_start(ones_t[:], ones_d[:])
                    nc.sync.dma_start(bv_t[:], bv_d[:])
                if g == 1:
                    nc.sync.dma_start(bk_t[:], bk_d[:])
                    nc.sync.dma_start(bq_t[:], bq_d[:])
                    nc.sync.dma_start(ones8_t[:], ones8_d[:])
            nc.sync.dma_start(wv_t[:], wv_d[:])

            kh = khp.tile([128, 8192], BF16)   # KhT: chunk hp at cols hp*2048
            vx = vxp.tile([128, 8320], BF16)   # V_ext: kv chunk jk at cols jk*520, head h at +h*65

            with (
                tc.tile_pool(name="ps1k", bufs=1, space="PSUM") as ps1k,
                tc.tile_pool(name="ps1v", bufs=1, space="PSUM") as ps1v,
            ):
                # bv broadcast across partitions (K=1 matmul with ones column)
                pb = ps1v.tile([128, 1024], F32, tag="v01", name="pbv")
                nc.tensor.matmul(pb[:, 0:512], ones_t[0:1, :], bv_t[:],
                                 start=True, stop=True)
                bvbc = const.tile([128, 512], F32, tag="bvbc")
                nc.vector.tensor_copy(bvbc[:], pb[:, 0:512])

                # ---- Phase 1: K then V projection per kv block. The K pass
                # (dedicated banks) runs while the previous block's V results
                # drain on the DVE, and vice versa, so the accumulator WARs at
                # block boundaries always have a full pass worth of slack.
                for kb in range(4):
                    if kb == 0:
                        ks = ks0
                    else:
                        ks = stream.tile([128, 4096], BF16, tag="stream", name="ks")
                        nc.sync.dma_start(ks[:], kvt_d[kb])
                        if kb == 1:
                            e2_t = const.tile([128, 128], F32R, tag="e2")
                            nc.sync.dma_start(e2_t[:], e2_d[:])
                            tri_t = const.tile([128, 128], BF16, tag="tri")
                            nc.sync.dma_start(tri_t[:], tri_d[:])
                            idn_t = const.tile([128, 128], BF16, tag="idn")
                            nc.sync.dma_start(idn_t[:], idn_d[:])
                            wq_t = wp.tile([128, 4096], BF16, tag="w")
                            nc.sync.dma_start(wq_t[:], wq_d[:])
                    pk01 = ps1k.tile([128, 1024], F32, tag="k01", name="pk01")
                    pk23 = ps1k.tile([128, 1024], F32, tag="k23", name="pk23")
                    pks = [pk01[:, 0:512], pk01[:, 512:1024],
                           pk23[:, 0:512], pk23[:, 512:1024]]
                    for c in range(8):
                        cs = slice(c * 512, (c + 1) * 512)
                        st, sp = (c == 0), (c == 7)
                        for mm in range(4):
                            nc.tensor.matmul(
                                pks[mm],
                                wk_t[:, c * 512 + mm * 128:c * 512 + (mm + 1) * 128],
                                ks[:, cs], start=st, stop=sp)
                    for mm in range(4):
                        nc.scalar.activation(
                            kh[:, mm * 2048 + kb * 512:mm * 2048 + (kb + 1) * 512],
                            pks[mm], AF.Identity, bias=bk_t[:, mm:mm + 1])
                    pv01 = ps1v.tile([128, 1024], F32, tag="v01", name="pv01")
                    pv23 = ps1v.tile([128, 1024], F32, tag="v23", name="pv23")
                    pvs = [pv01[:, 0:512], pv01[:, 512:1024],
                           pv23[:, 0:512], pv23[:, 512:1024]]
                    for c in range(8):
                        cs = slice(c * 512, (c + 1) * 512)
                        st, sp = (c == 0), (c == 7)
                        for js in range(4):
                            nc.tensor.matmul(
                                pvs[js],
                                ks[:, c * 512 + js * 128:c * 512 + (js + 1) * 128],
                                wv_t[:, cs], start=st, stop=sp)
                    for js in ((3, 2, 1, 0) if kb == 3 else (0, 1, 2, 3)):
                        jk = 4 * kb + js
                        dst = vx[:, jk * 520:(jk + 1) * 520].rearrange(
                            "p (h e) -> p h e", e=65)
                        nc.vector.tensor_tensor(
                            dst[:, :, 0:64],
                            pvs[js].rearrange("p (h e) -> p h e", e=64),
                            bvbc[:].rearrange("p (h e) -> p h e", e=64),
                            op=ALU.add)
                        nc.vector.tensor_copy(
                            dst[:, :, 64:65],
                            ones8_t[:].rearrange("p (h e) -> p h e", e=1))

            # prefetch the first two q blocks + wo while phase 1 drains;
            # stream pool has 3 bufs so these rotate behind ks2/ks3
            qs_tiles = {}

            def qs_load(qb):
                qs = stream.tile([128, 4096], BF16, tag="stream", name="qs")
                nc.sync.dma_start(qs[:], qt_d[qb])
                qs_tiles[qb] = qs

            qs_load(0)
            qs_load(1)
            qs_load(2)
            qs_load(3)
            wo_t = wp.tile([128, 4096], BF16, tag="w")
            nc.sync.dma_start(wo_t[:], wo_d[:])

            # ---- Phase 2: software-pipelined attention + Q/O projections.
            with (
                tc.tile_pool(name="ps_sc", bufs=1, space="PSUM") as ps_sc,
                tc.tile_pool(name="ps_out", bufs=1, space="PSUM") as ps_out,
                tc.tile_pool(name="ps_misc", bufs=2, space="PSUM") as ps_misc,
            ):
                def misc_ps(i, shape=(128, 512)):
                    return ps_misc.tile(list(shape), F32, tag="misc", name="miscp")

                qh_tiles = {}
                at_tiles = {}

                def qproj_start(qb):
                    qs = qs_tiles[qb]
                    qh = qhp.tile([128, 2048], BF16, name="qh")
                    qh_tiles[qb] = qh
                    return [(qproj_unit, (qb, qs, mm)) for mm in range(4)]

                def qproj_unit(qb, qs, mm, pp=None):
                    if pp is None:
                        pp = misc_ps(mm)
                    for c in range(8):
                        nc.tensor.matmul(
                            pp[:],
                            wq_t[:, c * 512 + mm * 128:c * 512 + (mm + 1) * 128],
                            qs[:, c * 512:(c + 1) * 512],
                            start=(c == 0), stop=(c == 7))
                    nc.vector.tensor_scalar_add(
                        qh_tiles[qb][:, mm * 512:(mm + 1) * 512],
                        pp[:], bq_t[:, mm:mm + 1])

                def oproj_unit(qb, u, pp=None):
                    ls, nb = u // 2, u % 2
                    tail = pp is not None
                    at = at_tiles[qb]
                    if pp is None:
                        pp = misc_ps(u)
                    for c4 in range(4):
                        nc.tensor.matmul(
                            pp[:],
                            at[:, c4 * 512 + ls * 128:c4 * 512 + (ls + 1) * 128],
                            wo_t[:, c4 * 1024 + nb * 512:c4 * 1024 + (nb + 1) * 512],
                            start=(c4 == 0), stop=(c4 == 3))
                    ot = osp.tile([128, 512], F32, name="ot")
                    if tail and u % 2:
                        nc.scalar.activation(ot[:], pp[:], AF.Identity)
                    else:
                        nc.vector.tensor_copy(ot[:], pp[:])
                    nc.sync.dma_start(
                        out_d[qb * 512 + ls * 128:qb * 512 + (ls + 1) * 128,
                              nb * 512:(nb + 1) * 512], ot[:])

                pending = []
                delayed = []
                deferred_norm = [None]

                def drain_one():
                    if pending:
                        fn, args = pending.pop(0)
                        fn(*args)

                upA = ps_sc.tile([128, 1024], F32, tag="scA", name="upA")
                upB = ps_sc.tile([128, 1024], F32, tag="scB", name="upB")
                upq = [upA[:, 0:512], upA[:, 512:1024],
                       upB[:, 0:512], upB[:, 512:1024]]
                for fn, args in qproj_start(0):
                    fn(*args, upq[args[2]])   # qb=0 Q-proj runs up front

                for qb in range(4):
                    if qb < 3:
                        pending.extend(qproj_start(qb + 1))
                    qh = qh_tiles[qb]
                    at = atp.tile([128, 2048], BF16, name="at")
                    at_tiles[qb] = at
                    nkv = 4 * qb + 4
                    rec0 = smp.tile([128, 512], F32R, tag="recs0", name="rec0")
                    rec1 = smp.tile([128, 512], F32R, tag="recs1", name="rec1")
                    npair = nkv // 2          # = 2*qb + 2 (last 2 are diagonal)
                    qb_slots = 4 * npair
                    qb_units = len(pending)
                    slot = 0
                    drained = 0
                    for hp in range(4):
                        # head pair (h0, h1) shares feature chunk hp; h0 uses PE
                        # row strips 0-63, h1 64-127.
                        h0, h1 = 2 * hp, 2 * hp + 1
                        opx = ps_out.tile([65, 1024], F32, tag="op")
                        op0 = opx[:, 0:512]
                        op1 = opx[:, 512:1024]
                        for jp in range(npair):
                            diag = jp >= 2 * qb   # last two pairs touch the diagonal
                            jk0, jk1 = 2 * jp, 2 * jp + 1
                            # h0's chunk pair in scA, h1's in scB: split tiles
                            # let scores(jp+1) on scA overlap exp(scB)
                            scA = ps_sc.tile([128, 1024], F32, tag="scA")
                            scB = ps_sc.tile([128, 1024], F32, tag="scB")
                            prA = prp.tile([128, 1024], BF16, tag="prA")
                            prB = prp.tile([128, 1024], BF16, tag="prB")
                            if not diag:
                                for half, jk in ((0, jk0), (1, jk1)):
                                    kcol = hp * 2048 + jk * 128
                                    hs = slice(half * 512, (half + 1) * 512)
                                    nc.tensor.matmul(
                                        scA[:, hs],
                                        kh[0:64, kcol:kcol + 128],
                                        qh[0:64, hp * 512:(hp + 1) * 512],
                                        start=True, stop=True)
                                    nc.tensor.matmul(
                                        scB[:, hs],
                                        kh[64:128, kcol:kcol + 128],
                                        qh[64:128, hp * 512:(hp + 1) * 512],
                                        start=True, stop=True)
                            else:
                                # diagonal pair: chunk t = jk - 4*qb is live only
                                # for local q >= 128*t; compute the live q range
                                # and add the causal triangle on the PE (bf16
                                # identity x triangle matmul into the same PSUM
                                # accumulation group).
                                for half, jk in ((0, jk0), (1, jk1)):
                                    t = jk - 4 * qb
                                    o = 128 * t
                                    kcol = hp * 2048 + jk * 128
                                    for sc_, r0, r1 in ((scA, 0, 64), (scB, 64, 128)):
                                        nc.tensor.matmul(
                                            sc_[:, half * 512 + o:(half + 1) * 512],
                                            kh[r0:r1, kcol:kcol + 128],
                                            qh[r0:r1, hp * 512 + o:(hp + 1) * 512],
                                            start=True, stop=False)
                                        nc.tensor.matmul(
                                            sc_[:, half * 512 + o:half * 512 + o + 128],
                                            idn_t[:], tri_t[:],
                                            start=False, stop=True)
                            if jp == 0 and deferred_norm[0] is not None:
                                # previous pair's normalization drops in here,
                                # after this pair's scores are already queued
                                deferred_norm[0]()
                                deferred_norm[0] = None
                            slot += 1
                            while pending and drained * qb_slots < slot * qb_units:
                                drain_one()
                                drained += 1
                            first, last = (jp == 0), (jp == npair - 1)
                            if not diag:
                                nc.scalar.activation(prA[:], scA[:], AF.Exp)
                                nc.scalar.activation(prB[:], scB[:], AF.Exp)
                                for pr_, op_, h_ in ((prA, op0, h0), (prB, op1, h1)):
                                    nc.tensor.matmul(
                                        op_[:],
                                        vx[:, jk0 * 520 + h_ * 65:jk0 * 520 + (h_ + 1) * 65],
                                        pr_[:, 0:512], start=first, stop=False)
                                    nc.tensor.matmul(
                                        op_[:],
                                        vx[:, jk1 * 520 + h_ * 65:jk1 * 520 + (h_ + 1) * 65],
                                        pr_[:, 512:1024], start=False, stop=False)
                            else:
                                for half, jk in ((0, jk0), (1, jk1)):
                                    t = jk - 4 * qb
                                    o = 128 * t
                                    es = slice(half * 512 + o, (half + 1) * 512)
                                    nc.scalar.activation(prA[:, es], scA[:, es], AF.Exp)
                                    nc.scalar.activation(prB[:, es], scB[:, es], AF.Exp)
                                for half, jk in ((0, jk0), (1, jk1)):
                                    t = jk - 4 * qb
                                    o = 128 * t
                                    es = slice(half * 512 + o, (half + 1) * 512)
                                    st = first and half == 0
                                    sp = last and half == 1
                                    for pr_, op_, h_ in ((prA, op0, h0), (prB, op1, h1)):
                                        nc.tensor.matmul(
                                            op_[:, o:512],
                                            vx[:, jk * 520 + h_ * 65:jk * 520 + (h_ + 1) * 65],
                                            pr_[:, es], start=st, stop=sp)

                        def emit_norm(hp=hp, at=at, opx=opx, rec0=rec0, rec1=rec1):
                            # reciprocals (DVE; quadrant rows: partition bases
                            # must be 32-aligned) + stash copies free opx for
                            # the next head pair; the normalize itself is
                            # appended as a filler unit so its PE broadcast
                            # matmul never waits on the reciprocal in-loop
                            with nc.allow_low_precision(reason="f32r recip for normalize"):
                                for par, rec in ((0, rec0), (1, rec1)):
                                    nc.vector.reciprocal(
                                        rec[32 * hp:32 * hp + 1, :],
                                        opx[64:65, par * 512:(par + 1) * 512])
                            for par in (0, 1):
                                dst = at[64 * par:64 * par + 64,
                                         hp * 512:(hp + 1) * 512]
                                src_ = opx[0:64, par * 512:(par + 1) * 512]
                                if qb == 3 and hp == 3:
                                    nc.scalar.activation(dst, src_, AF.Identity)
                                else:
                                    nc.vector.tensor_copy(dst, src_)
                            pending.append((norm_unit, (hp, at, rec0, rec1)))

                        def norm_unit(hp, at, rec0, rec1):
                            for par, rec in ((0, rec0), (1, rec1)):
                                bc = misc_ps(par, shape=(64, 512))
                                nc.tensor.matmul(
                                    bc[:], e2_t[32 * hp:32 * hp + 1, 0:64],
                                    rec[32 * hp:32 * hp + 1, :],
                                    start=True, stop=True,
                                    tile_position=(32 * hp, 0))
                                atsl = at[64 * par:64 * par + 64,
                                          hp * 512:(hp + 1) * 512]
                                nc.vector.tensor_tensor(atsl, atsl, bc[:], op=ALU.mult)
                        deferred_norm[0] = emit_norm
                    if deferred_norm[0] is not None:
                        deferred_norm[0]()
                        deferred_norm[0] = None
                    units = [(oproj_unit, (qb, u)) for u in range(8)]
                    # o-proj of block qb drains during qb+2 (qb+1 has q-proj
                    # fillers already; qb=3 would otherwise starve)
                    if qb == 0:
                        delayed = units
                    elif qb == 1:
                        pending.extend(delayed)
                        delayed = units
                    elif qb == 2:
                        pending.extend(delayed)
                        pending.extend(units)
                    else:
                        pending.extend(units)

                # tail: the final q block's O-projection drains with nothing to
                # hide behind; rotate its accumulators through the freed 4-bank
                # sc tile (quarters) so the mm->copy->dma pipeline stays deep
                tail = [(fn, args) for fn, args in pending]
                pending.clear()
                tailA = ps_sc.tile([128, 1024], F32, tag="scA", name="tailA")
                tailB = ps_sc.tile([128, 1024], F32, tag="scB", name="tailB")
                quarters = [tailA[:, 0:512], tailA[:, 512:1024],
                            tailB[:, 0:512], tailB[:, 512:1024]]
                for i, (fn, args) in enumerate(tail):
                    if fn is oproj_unit and args[0] == 3:
                        u = args[1]
                        fn(args[0], u, quarters[u % 4])
                    else:
                        fn(*args)

    nc.compile()
    return nc


def _get_nc():
    if "nc" not in _CACHE:
        _CACHE["nc"] = _build_nc()
    return _CACHE["nc"]


def _prep_w(Wg):
    # W_g.T [1024, 512] -> [128, 4096]: col (c, n) -> c*512 + n; row p = k within chunk
    return np.ascontiguousarray(
        Wg.T.reshape(8, 128, 512).transpose(1, 0, 2).reshape(128, 4096)
    ).astype(ml_dtypes.bfloat16)


def _prep_seqT(x):
    # x [2048, 1024] -> [4, 128, 4096]: [blk][p][c*512 + j] = x[blk*512 + j, c*128 + p]
    return np.ascontiguousarray(
        x.reshape(4, 512, 8, 128).transpose(0, 3, 2, 1).reshape(4, 128, 4096)
    ).astype(ml_dtypes.bfloat16)


def _tri():
    # triangle mask for a 128x128 diagonal block: allow kv row p for local
    # col ql iff p <= ql
    p = np.arange(128)[:, None]
    ql = np.arange(128)[None, :]
    return np.where(p <= ql, 0.0, -10000.0).astype(ml_dtypes.bfloat16)


def _prep_wo(Wog):
    # Wo[:, g] slice transposed: [512, 1024] -> [128, 4096] col (c, nb, n) -> c*1024 + nb*512 + n
    return np.ascontiguousarray(
        Wog.T.reshape(4, 128, 2, 512).transpose(1, 0, 2, 3).reshape(128, 4096)
    ).astype(ml_dtypes.bfloat16)


def kernel(**inputs):
    from concourse.bass_utils import run_bass_kernel_spmd

    kv = np.asarray(inputs["kv"], np.float32)
    q = np.asarray(inputs["q"], np.float32)
    Wq = np.asarray(inputs["Wq"], np.float32)
    bq = np.asarray(inputs["bq"], np.float32)
    Wk = np.asarray(inputs["Wk"], np.float32)
    bk = np.asarray(inputs["bk"], np.float32)
    Wv = np.asarray(inputs["Wv"], np.float32)
    bv = np.asarray(inputs["bv"], np.float32)
    Wo = np.asarray(inputs["Wo"], np.float32)
    bo = np.asarray(inputs["bo"], np.float32)

    nc = _get_nc()
    ones = np.ones((1, 128), np.float32)
    ones8 = np.ones((128, 8), np.float32)
    tri = _tri()
    idn = np.eye(128).astype(ml_dtypes.bfloat16)
    e2 = np.zeros((128, 128), np.float32)
    for c in range(4):
        e2[32 * c + 0, 0:64] = 1.0
        e2[32 * c + 1, 64:128] = 1.0

    in_maps = []
    for c in range(NCORES):
        b, g = c // 2, c % 2
        sl = slice(g * 512, (g + 1) * 512)
        in_maps.append({
            "qt": _prep_seqT(q[b]),
            "kvt": _prep_seqT(kv[b]),
            "wq": _prep_w(Wq[sl] * SCALE),
            "wk": _prep_w(Wk[sl]),
            "wv": _prep_w(Wv[sl]),
            "wo": _prep_wo(Wo[:, sl]),
            "bq": np.ascontiguousarray((bq[sl] * SCALE).reshape(4, 128).T),
            "bk": np.ascontiguousarray(bk[sl].reshape(4, 128).T),
            "bv": bv[sl].reshape(1, 512),
            "ones": ones,
            "ones8": ones8,
            "tri": tri,
            "idn": idn,
            "e2": e2,
        })

    res = run_bass_kernel_spmd(nc, in_maps, core_ids=list(range(NCORES)),
                               **_CACHE.get("run_kwargs", {}))
    _CACHE["last_results"] = res
    out = np.empty((B, L, D), np.float32)
    for b in range(B):
        out[b] = res.results[2 * b]["out"] + res.results[2 * b + 1]["out"] + bo[None, :]
    return out


# revision 13
# speedup vs baseline: 1.0314x; 1.0314x over previous
"""Causal cross-attention (B=4, L=2048, D=1024, H=16, hd=64) on 8 trn2 cores.

Sharding: core c -> (batch b = c//2, head-group g = c%2 of 8 heads).
Each core computes QKV projections for its head group, causal-masked
per-head attention, and a partial output projection (its heads' columns
of Wo). Host sums the two partials per batch and adds bo.

On-chip layout is feature-major ("transposed") end to end so no on-chip
transposes are needed:
  QhT/KhT: [feat, seq], V: [seq, feat(+ones col)] -> scoresT = KhT_blk.T @ QhT_blk
  attn_outT accumulated as V_ext.T @ probsT with a ones column giving the
  softmax denominator for free; normalization via reciprocal + K=1
  broadcast matmul.

v2 scheduling notes:
  - Causal masking runs on the PE as a tiny bf16 matmul (identity x
    triangle) accumulated into the scores PSUM group - the DVE is out of
    the scores->exp critical path entirely.
  - Diagonal kv chunks compute only the live q range (512/384/256/128
    wide); qh and the probs are bf16 so narrow matmuls still stream at
    1 cycle/row.
  - PSUM->SBUF staging copies (attention out, o-proj out, v bias) run on
    the otherwise idle GPSIMD/Pool engine.
  - Phase 1 (K/V projection) walks the contraction dim chunk-major with
    8 PSUM accumulator banks so compute starts after the first 512-col
    chunk of wk/kv arrives; DMAs are issued in consumption order.
"""

import numpy as np
import ml_dtypes

B, L, D, H, HD = 4, 2048, 1024, 16, 64
NCORES = 8
SCALE = HD ** -0.5

_CACHE = {}


def _build_nc():
    import concourse.mybir as mybir
    import concourse.tile as tile
    from concourse import bacc

    F32 = mybir.dt.float32
    F32R = mybir.dt.float32r
    BF16 = mybir.dt.bfloat16
    AF = mybir.ActivationFunctionType
    ALU = mybir.AluOpType

    nc = bacc.Bacc("TRN2", target_bir_lowering=False, debug=False)

    qt_d = nc.declare_dram_parameter("qt", [4, 128, 4096], BF16, isOutput=False)
    kvt_d = nc.declare_dram_parameter("kvt", [4, 128, 4096], BF16, isOutput=False)
    wq_d = nc.declare_dram_parameter("wq", [128, 4096], BF16, isOutput=False)
    wk_d = nc.declare_dram_parameter("wk", [128, 4096], BF16, isOutput=False)
    wv_d = nc.declare_dram_parameter("wv", [128, 4096], BF16, isOutput=False)
    wo_d = nc.declare_dram_parameter("wo", [128, 4096], BF16, isOutput=False)
    bq_d = nc.declare_dram_parameter("bq", [128, 4], F32, isOutput=False)
    bk_d = nc.declare_dram_parameter("bk", [128, 4], F32, isOutput=False)
    bv_d = nc.declare_dram_parameter("bv", [1, 512], F32R, isOutput=False)
    ones_d = nc.declare_dram_parameter("ones", [1, 128], F32R, isOutput=False)
    ones8_d = nc.declare_dram_parameter("ones8", [128, 8], F32R, isOutput=False)
    tri_d = nc.declare_dram_parameter("tri", [128, 128], BF16, isOutput=False)
    idn_d = nc.declare_dram_parameter("idn", [128, 128], BF16, isOutput=False)
    e2_d = nc.declare_dram_parameter("e2", [128, 128], F32R, isOutput=False)
    out_d = nc.declare_dram_parameter("out", [2048, 1024], F32, isOutput=True)

    with tile.TileContext(nc) as tc:
        with (
            tc.tile_pool(name="const", bufs=1) as const,
            tc.tile_pool(name="w", bufs=4) as wp,
            tc.tile_pool(name="stream", bufs=4) as stream,
            tc.tile_pool(name="khp", bufs=1) as khp,
            tc.tile_pool(name="vxp", bufs=1) as vxp,
            tc.tile_pool(name="qhp", bufs=2) as qhp,
            tc.tile_pool(name="atp", bufs=2) as atp,
            tc.tile_pool(name="prp", bufs=2) as prp,
            tc.tile_pool(name="smp", bufs=2) as smp,
            tc.tile_pool(name="osp", bufs=4) as osp,
        ):
            # consts needed by the very first PE op (bv broadcast) go first;
            # the rest of the small consts slot in behind the first two
            # wk/kv chunk loads so they don't delay compute start
            bq_t = const.tile([128, 4], F32, tag="bq")
            bk_t = const.tile([128, 4], F32, tag="bk")
            ones_t = const.tile([1, 128], F32R, tag="ones")
            ones8_t = const.tile([128, 8], F32R, tag="ones8")
            bv_t = const.tile([1, 512], F32R, tag="bv")
            wk_t = wp.tile([128, 4096], BF16, tag="w")
            wv_t = wp.tile([128, 4096], BF16, tag="w")
            ks0 = stream.tile([128, 4096], BF16, tag="stream", name="ks0")
            for g in range(4):
                s = slice(g * 1024, (g + 1) * 1024)
                nc.sync.dma_start(wk_t[:, s], wk_d[:, s])
                nc.sync.dma_start(ks0[:, s], kvt_d[0, :, s])
                if g == 0:
                    nc.sync.dma_start(ones_t[:], ones_d[:])
                    nc.sync.dma_start(bv_t[:], bv_d[:])
                if g == 1:
                    nc.sync.dma_start(bk_t[:], bk_d[:])
                    nc.sync.dma_start(bq_t[:], bq_d[:])
                    nc.sync.dma_start(ones8_t[:], ones8_d[:])
            nc.sync.dma_start(wv_t[:], wv_d[:])

            kh = khp.tile([128, 8192], BF16)   # KhT: chunk hp at cols hp*2048
            vx = vxp.tile([128, 8320], BF16)   # V_ext: kv chunk jk at cols jk*520, head h at +h*65

            with (
                tc.tile_pool(name="ps1k", bufs=1, space="PSUM") as ps1k,
                tc.tile_pool(name="ps1v", bufs=1, space="PSUM") as ps1v,
            ):
                # bv broadcast across partitions (K=1 matmul with ones column)
                pb = ps1v.tile([128, 1024], F32, tag="v01", name="pbv")
                nc.tensor.matmul(pb[:, 0:512], ones_t[0:1, :], bv_t[:],
                                 start=True, stop=True)
                bvbc = const.tile([128, 512], F32, tag="bvbc")
                nc.vector.tensor_copy(bvbc[:], pb[:, 0:512])

                # ---- Phase 1: K then V projection per kv block. The K pass
                # (dedicated banks) runs while the previous block's V results
                # drain on the DVE, and vice versa, so the accumulator WARs at
                # block boundaries always have a full pass worth of slack.
                for kb in range(4):
                    if kb == 0:
                        ks = ks0
                    else:
                        ks = stream.tile([128, 4096], BF16, tag="stream", name="ks")
                        nc.sync.dma_start(ks[:], kvt_d[kb])
                        if kb == 1:
                            e2_t = const.tile([128, 128], F32R, tag="e2")
                            nc.sync.dma_start(e2_t[:], e2_d[:])
                            tri_t = const.tile([128, 128], BF16, tag="tri")
                            nc.sync.dma_start(tri_t[:], tri_d[:])
                            idn_t = const.tile([128, 128], BF16, tag="idn")
                            nc.sync.dma_start(idn_t[:], idn_d[:])
                            wq_t = wp.tile([128, 4096], BF16, tag="w")
                            nc.sync.dma_start(wq_t[:], wq_d[:])
                    pk01 = ps1k.tile([128, 1024], F32, tag="k01", name="pk01")
                    pk23 = ps1k.tile([128, 1024], F32, tag="k23", name="pk23")
                    pks = [pk01[:, 0:512], pk01[:, 512:1024],
                           pk23[:, 0:512], pk23[:, 512:1024]]
                    for c in range(8):
                        cs = slice(c * 512, (c + 1) * 512)
                        st, sp = (c == 0), (c == 7)
                        for mm in range(4):
                            nc.tensor.matmul(
                                pks[mm],
                                wk_t[:, c * 512 + mm * 128:c * 512 + (mm + 1) * 128],
                                ks[:, cs], start=st, stop=sp)
                    for mm in range(4):
                        nc.scalar.activation(
                            kh[:, mm * 2048 + kb * 512:mm * 2048 + (kb + 1) * 512],
                            pks[mm], AF.Identity, bias=bk_t[:, mm:mm + 1])
                    pv01 = ps1v.tile([128, 1024], F32, tag="v01", name="pv01")
                    pv23 = ps1v.tile([128, 1024], F32, tag="v23", name="pv23")
                    pvs = [pv01[:, 0:512], pv01[:, 512:1024],
                           pv23[:, 0:512], pv23[:, 512:1024]]
                    for c in range(8):
                        cs = slice(c * 512, (c + 1) * 512)
                        st, sp = (c == 0), (c == 7)
                        for js in range(4):
                            nc.tensor.matmul(
                                pvs[js],
                                ks[:, c * 512 + js * 128:c * 512 + (js + 1) * 128],
                                wv_t[:, cs], start=st, stop=sp)
                    for js in ((3, 2, 1, 0) if kb == 3 else (0, 1, 2, 3)):
                        jk = 4 * kb + js
                        dst = vx[:, jk * 520:(jk + 1) * 520].rearrange(
                            "p (h e) -> p h e", e=65)
                        nc.vector.tensor_tensor(
                            dst[:, :, 0:64],
                            pvs[js].rearrange("p (h e) -> p h e", e=64),
                            bvbc[:].rearrange("p (h e) -> p h e", e=64),
                            op=ALU.add)
                        nc.vector.tensor_copy(
                            dst[:, :, 64:65],
                            ones8_t[:].rearrange("p (h e) -> p h e", e=1))

            # prefetch the first two q blocks + wo while phase 1 drains;
            # stream pool has 3 bufs so these rotate behind ks2/ks3
            qs_tiles = {}

            def qs_load(qb):
                qs = stream.tile([128, 4096], BF16, tag="stream", name="qs")
                nc.sync.dma_start(qs[:], qt_d[qb])
                qs_tiles[qb] = qs

            qs_load(0)
            qs_load(1)
            qs_load(2)
            qs_load(3)
            wo_t = wp.tile([128, 4096], BF16, tag="w")
            nc.sync.dma_start(wo_t[:], wo_d[:])

            # ---- Phase 2: software-pipelined attention + Q/O projections.
            with (
                tc.tile_pool(name="ps_sc", bufs=1, space="PSUM") as ps_sc,
                tc.tile_pool(name="ps_out", bufs=1, space="PSUM") as ps_out,
                tc.tile_pool(name="ps_misc", bufs=2, space="PSUM") as ps_misc,
            ):
                def misc_ps(i, shape=(128, 512)):
                    return ps_misc.tile(list(shape), F32, tag="misc", name="miscp")

                qh_tiles = {}
                at_tiles = {}

                def qproj_start(qb):
                    qs = qs_tiles[qb]
                    qh = qhp.tile([128, 2048], BF16, name="qh")
                    qh_tiles[qb] = qh
                    return [(qproj_unit, (qb, qs, mm)) for mm in range(4)]

                def qproj_unit(qb, qs, mm, pp=None):
                    if pp is None:
                        pp = misc_ps(mm)
                    for c in range(8):
                        nc.tensor.matmul(
                            pp[:],
                            wq_t[:, c * 512 + mm * 128:c * 512 + (mm + 1) * 128],
                            qs[:, c * 512:(c + 1) * 512],
                            start=(c == 0), stop=(c == 7))
                    nc.vector.tensor_scalar_add(
                        qh_tiles[qb][:, mm * 512:(mm + 1) * 512],
                        pp[:], bq_t[:, mm:mm + 1])

                def oproj_unit(qb, u, pp=None):
                    ls, nb = u // 2, u % 2
                    tail = pp is not None
                    at = at_tiles[qb]
                    if pp is None:
                        pp = misc_ps(u)
                    for c4 in range(4):
                        nc.tensor.matmul(
                            pp[:],
                            at[:, c4 * 512 + ls * 128:c4 * 512 + (ls + 1) * 128],
                            wo_t[:, c4 * 1024 + nb * 512:c4 * 1024 + (nb + 1) * 512],
                            start=(c4 == 0), stop=(c4 == 3))
                    ot = osp.tile([128, 512], F32, name="ot")
                    if tail and u % 2:
                        nc.scalar.activation(ot[:], pp[:], AF.Identity)
                    else:
                        nc.vector.tensor_copy(ot[:], pp[:])
                    nc.sync.dma_start(
                        out_d[qb * 512 + ls * 128:qb * 512 + (ls + 1) * 128,
                              nb * 512:(nb + 1) * 512], ot[:])

                pending = []
                delayed = []
                deferred_norm = [None]

                def drain_one():
                    if pending:
                        fn, args = pending.pop(0)
                        fn(*args)

                upA = ps_sc.tile([128, 1024], F32, tag="scA", name="upA")
                upB = ps_sc.tile([128, 1024], F32, tag="scB", name="upB")
                upq = [upA[:, 0:512], upA[:, 512:1024],
                       upB[:, 0:512], upB[:, 512:1024]]
                for fn, args in qproj_start(0):
                    fn(*args, upq[args[2]])   # qb=0 Q-proj runs up front

                for qb in range(4):
                    if qb < 3:
                        pending.extend(qproj_start(qb + 1))
                    qh = qh_tiles[qb]
                    at = atp.tile([128, 2048], BF16, name="at")
                    at_tiles[qb] = at
                    nkv = 4 * qb + 4
                    rec0 = smp.tile([128, 512], F32R, tag="recs0", name="rec0")
                    rec1 = smp.tile([128, 512], F32R, tag="recs1", name="rec1")
                    npair = nkv // 2          # = 2*qb + 2 (last 2 are diagonal)
                    qb_slots = 4 * npair
                    qb_units = len(pending)
                    slot = 0
                    drained = 0
                    for hp in range(4):
                        # head pair (h0, h1) shares feature chunk hp; h0 uses PE
                        # row strips 0-63, h1 64-127.
                        h0, h1 = 2 * hp, 2 * hp + 1
                        opx = ps_out.tile([65, 1024], F32, tag="op")
                        op0 = opx[:, 0:512]
                        op1 = opx[:, 512:1024]
                        for jp in range(npair):
                            diag = jp >= 2 * qb   # last two pairs touch the diagonal
                            jk0, jk1 = 2 * jp, 2 * jp + 1
                            # h0's chunk pair in scA, h1's in scB: split tiles
                            # let scores(jp+1) on scA overlap exp(scB)
                            scA = ps_sc.tile([128, 1024], F32, tag="scA")
                            scB = ps_sc.tile([128, 1024], F32, tag="scB")
                            prA = prp.tile([128, 1024], BF16, tag="prA")
                            prB = prp.tile([128, 1024], BF16, tag="prB")
                            if not diag:
                                for half, jk in ((0, jk0), (1, jk1)):
                                    kcol = hp * 2048 + jk * 128
                                    hs = slice(half * 512, (half + 1) * 512)
                                    nc.tensor.matmul(
                                        scA[:, hs],
                                        kh[0:64, kcol:kcol + 128],
                                        qh[0:64, hp * 512:(hp + 1) * 512],
                                        start=True, stop=True)
                                    nc.tensor.matmul(
                                        scB[:, hs],
                                        kh[64:128, kcol:kcol + 128],
                                        qh[64:128, hp * 512:(hp + 1) * 512],
                                        start=True, stop=True)
                            else:
                                # diagonal pair: chunk t = jk - 4*qb is live only
                                # for local q >= 128*t; compute the live q range
                                # and add the causal triangle on the PE (bf16
                                # identity x triangle matmul into the same PSUM
                                # accumulation group).
                                for half, jk in ((0, jk0), (1, jk1)):
                                    t = jk - 4 * qb
                                    o = 128 * t
                                    kcol = hp * 2048 + jk * 128
                                    for sc_, r0, r1 in ((scA, 0, 64), (scB, 64, 128)):
                                        nc.tensor.matmul(
                                            sc_[:, half * 512 + o:(half + 1) * 512],
                                            kh[r0:r1, kcol:kcol + 128],
                                            qh[r0:r1, hp * 512 + o:(hp + 1) * 512],
                                            start=True, stop=False)
                                        nc.tensor.matmul(
                                            sc_[:, half * 512 + o:half * 512 + o + 128],
                                            idn_t[:], tri_t[:],
                                            start=False, stop=True)
                            if jp == 0 and deferred_norm[0] is not None:
                                # previous pair's normalization drops in here,
                                # after this pair's scores are already queued
                                deferred_norm[0]()
                                deferred_norm[0] = None
                            slot += 1
                            while pending and drained * qb_slots < slot * qb_units:
                                drain_one()
                                drained += 1
                            first, last = (jp == 0), (jp == npair - 1)
                            if not diag:
                                nc.scalar.activation(prA[:], scA[:], AF.Exp)
                                nc.scalar.activation(prB[:], scB[:], AF.Exp)
                                for pr_, op_, h_ in ((prA, op0, h0), (prB, op1, h1)):
                                    nc.tensor.matmul(
                                        op_[:],
                                        vx[:, jk0 * 520 + h_ * 65:jk0 * 520 + (h_ + 1) * 65],
                                        pr_[:, 0:512], start=first, stop=False)
                                    nc.tensor.matmul(
                                        op_[:],
                                        vx[:, jk1 * 520 + h_ * 65:jk1 * 520 + (h_ + 1) * 65],
                                        pr_[:, 512:1024], start=False, stop=False)
                            else:
                                for half, jk in ((0, jk0), (1, jk1)):
                                    t = jk - 4 * qb
                                    o = 128 * t
                                    es = slice(half * 512 + o, (half + 1) * 512)
                                    nc.scalar.activation(prA[:, es], scA[:, es], AF.Exp)
                                    nc.scalar.activation(prB[:, es], scB[:, es], AF.Exp)
                                for half, jk in ((0, jk0), (1, jk1)):
                                    t = jk - 4 * qb
                                    o = 128 * t
                                    es = slice(half * 512 + o, (half + 1) * 512)
                                    st = first and half == 0
                                    sp = last and half == 1
                                    for pr_, op_, h_ in ((prA, op0, h0), (prB, op1, h1)):
                                        nc.tensor.matmul(
                                            op_[:, o:512],
                                            vx[:, jk * 520 + h_ * 65:jk * 520 + (h_ + 1) * 65],
                                            pr_[:, es], start=st, stop=sp)

                        def emit_norm(hp=hp, at=at, opx=opx, rec0=rec0, rec1=rec1):
                            # reciprocals (DVE; quadrant rows: partition bases
                            # must be 32-aligned) + stash copies free opx for
                            # the next head pair; the normalize itself is
                            # appended as a filler unit so its PE broadcast
                            # matmul never waits on the reciprocal in-loop
                            with nc.allow_low_precision(reason="f32r recip for normalize"):
                                for par, rec in ((0, rec0), (1, rec1)):
                                    nc.vector.reciprocal(
                                        rec[32 * hp:32 * hp + 1, :],
                                        opx[64:65, par * 512:(par + 1) * 512])
                            for par in (0, 1):
                                dst = at[64 * par:64 * par + 64,
                                         hp * 512:(hp + 1) * 512]
                                src_ = opx[0:64, par * 512:(par + 1) * 512]
                                if qb == 3 and hp == 3:
                                    nc.scalar.activation(dst, src_, AF.Identity)
                                else:
                                    nc.vector.tensor_copy(dst, src_)
                            pending.append((norm_unit, (hp, at, rec0, rec1)))

                        def norm_unit(hp, at, rec0, rec1):
                            for par, rec in ((0, rec0), (1, rec1)):
                                bc = misc_ps(par, shape=(64, 512))
                                nc.tensor.matmul(
                                    bc[:], e2_t[32 * hp:32 * hp + 1, 0:64],
                                    rec[32 * hp:32 * hp + 1, :],
                                    start=True, stop=True,
                                    tile_position=(32 * hp, 0))
                                atsl = at[64 * par:64 * par + 64,
                                          hp * 512:(hp + 1) * 512]
                                nc.vector.tensor_tensor(atsl, atsl, bc[:], op=ALU.mult)
                        deferred_norm[0] = emit_norm
                    if deferred_norm[0] is not None:
                        deferred_norm[0]()
                        deferred_norm[0] = None
                    pending.extend([(oproj_unit, (qb, u)) for u in range(8)])

                # tail: the final q block's O-projection drains with nothing to
                # hide behind; rotate its accumulators through the freed 4-bank
                # sc tile (quarters) so the mm->copy->dma pipeline stays deep
                tail = [(fn, args) for fn, args in pending]
                pending.clear()
                tailA = ps_sc.tile([128, 1024], F32, tag="scA", name="tailA")
                tailB = ps_sc.tile([128, 1024], F32, tag="scB", name="tailB")
                quarters = [tailA[:, 0:512], tailA[:, 512:1024],
                            tailB[:, 0:512], tailB[:, 512:1024]]
                for i, (fn, args) in enumerate(tail):
                    if fn is oproj_unit and args[0] == 3:
                        u = args[1]
                        fn(args[0], u, quarters[u % 4])
                    else:
                        fn(*args)

    nc.compile()
    return nc


def _get_nc():
    if "nc" not in _CACHE:
        _CACHE["nc"] = _build_nc()
    return _CACHE["nc"]


def _prep_w(Wg):
    # W_g.T [1024, 512] -> [128, 4096]: col (c, n) -> c*512 + n; row p = k within chunk
    return np.ascontiguousarray(
        Wg.T.reshape(8, 128, 512).transpose(1, 0, 2).reshape(128, 4096)
    ).astype(ml_dtypes.bfloat16)


def _prep_seqT(x):
    # x [2048, 1024] -> [4, 128, 4096]: [blk][p][c*512 + j] = x[blk*512 + j, c*128 + p]
    return np.ascontiguousarray(
        x.reshape(4, 512, 8, 128).transpose(0, 3, 2, 1).reshape(4, 128, 4096)
    ).astype(ml_dtypes.bfloat16)


def _tri():
    # triangle mask for a 128x128 diagonal block: allow kv row p for local
    # col ql iff p <= ql
    p = np.arange(128)[:, None]
    ql = np.arange(128)[None, :]
    return np.where(p <= ql, 0.0, -10000.0).astype(ml_dtypes.bfloat16)


def _prep_wo(Wog):
    # Wo[:, g] slice transposed: [512, 1024] -> [128, 4096] col (c, nb, n) -> c*1024 + nb*512 + n
    return np.ascontiguousarray(
        Wog.T.reshape(4, 128, 2, 512).transpose(1, 0, 2, 3).reshape(128, 4096)
    ).astype(ml_dtypes.bfloat16)


def kernel(**inputs):
    from concourse.bass_utils import run_bass_kernel_spmd

    kv = np.asarray(inputs["kv"], np.float32)
    q = np.asarray(inputs["q"], np.float32)
    Wq = np.asarray(inputs["Wq"], np.float32)
    bq = np.asarray(inputs["bq"], np.float32)
    Wk = np.asarray(inputs["Wk"], np.float32)
    bk = np.asarray(inputs["bk"], np.float32)
    Wv = np.asarray(inputs["Wv"], np.float32)
    bv = np.asarray(inputs["bv"], np.float32)
    Wo = np.asarray(inputs["Wo"], np.float32)
    bo = np.asarray(inputs["bo"], np.float32)

    nc = _get_nc()
    ones = np.ones((1, 128), np.float32)
    ones8 = np.ones((128, 8), np.float32)
    tri = _tri()
    idn = np.eye(128).astype(ml_dtypes.bfloat16)
    e2 = np.zeros((128, 128), np.float32)
    for c in range(4):
        e2[32 * c + 0, 0:64] = 1.0
        e2[32 * c + 1, 64:128] = 1.0

    in_maps = []
    for c in range(NCORES):
        b, g = c // 2, c % 2
        sl = slice(g * 512, (g + 1) * 512)
        in_maps.append({
            "qt": _prep_seqT(q[b]),
            "kvt": _prep_seqT(kv[b]),
            "wq": _prep_w(Wq[sl] * SCALE),
            "wk": _prep_w(Wk[sl]),
            "wv": _prep_w(Wv[sl]),
            "wo": _prep_wo(Wo[:, sl]),
            "bq": np.ascontiguousarray((bq[sl] * SCALE).reshape(4, 128).T),
            "bk": np.ascontiguousarray(bk[sl].reshape(4, 128).T),
            "bv": bv[sl].reshape(1, 512),
            "ones": ones,
            "ones8": ones8,
            "tri": tri,
            "idn": idn,
            "e2": e2,
        })

    res = run_bass_kernel_spmd(nc, in_maps, core_ids=list(range(NCORES)),
                               **_CACHE.get("run_kwargs", {}))
    _CACHE["last_results"] = res
    out = np.empty((B, L, D), np.float32)
    for b in range(B):
        out[b] = res.results[2 * b]["out"] + res.results[2 * b + 1]["out"] + bo[None, :]
    return out
